# revision 25
# baseline (speedup 1.0000x reference)
"""Trainium2 Bass kernel: RMSNorm + QKV + YaRN RoPE + sliding-window GQA attention
with sink logits + output projection + residual.

Sharding: data-parallel over batch (2) x tensor-parallel over KV-head pairs (4).
Each of the 8 cores computes, for one batch element and 2 of the 8 KV heads
(16 of the 64 Q heads), the full fused block and a partial output projection.
The host sums the 4 partial projections per batch and adds bias + residual.

All device matmuls run as float32r (full-rate fp32 path on the PE array).

Weight/layout tricks done on host (all exact):
  - norm_w folded into qkv_w columns; softmax scale 1/8 folded into Q rows+bias.
  - Q/K rows permuted so RoPE halves (i, i+32) sit in adjacent partitions
    (2i, 2i+1) -> rotate-half becomes an intra-quadrant stream_shuffle.
  - Weights pre-transposed/blocked for contiguous [K-part, M] lhsT tiles.
  - x pre-transposed to [hidden, token]; hidden zero-padded 2880 -> 2944.
"""

import numpy as np

import concourse.bass as bass
import concourse.tile as tile
from concourse import bacc, mybir
from concourse.bass_utils import run_bass_kernel_spmd

# problem constants
B, SEQ, HID = 2, 1024, 2880
NH, NKV, D = 64, 8, 64
HIDP = 2944            # 23 * 128
KT = HIDP // 128       # 23 hidden k-tiles
QKV_M = 10             # 1280 rows per core / 128
OUT_M = KT             # output hidden tiles (padded)
OUT_K = 8              # 1024 attn features / 128
NT = SEQ               # tokens per core
CH = 512               # matmul moving chunk
EPS = 1e-5
MASK_NEG = -100.0

F32 = mybir.dt.float32
F32R = mybir.dt.float32r

PAIR_SWAP = [i ^ 1 for i in range(32)]


# ---------------------------------------------------------------- device code
def build_nc():
    nc = bacc.Bacc("TRN2", target_bir_lowering=False, debug=False)

    xt_d = nc.dram_tensor("xt", [KT, 128, NT], F32R, kind="ExternalInput")
    wqkv_d = nc.dram_tensor("wqkv", [QKV_M, 128, KT * 128], F32R, kind="ExternalInput")
    bqkv_d = nc.dram_tensor("bqkv", [128, QKV_M], F32, kind="ExternalInput")
    wout_d = nc.dram_tensor("wout", [OUT_M, 128, OUT_K * 128], F32R, kind="ExternalInput")
    cos_d = nc.dram_tensor("cos128", [128, NT], F32, kind="ExternalInput")
    sin_d = nc.dram_tensor("sin128", [128, NT], F32, kind="ExternalInput")
    maskd_d = nc.dram_tensor("maskd", [128, CH], F32, kind="ExternalInput")
    maskl_d = nc.dram_tensor("maskl", [128, CH], F32, kind="ExternalInput")
    sink_d = nc.dram_tensor("sinkrow", [1, 4 * CH], F32, kind="ExternalInput")
    id_d = nc.dram_tensor("ident64", [128, 64], F32, kind="ExternalInput")
    ones_d = nc.dram_tensor("ones128", [128, 1], F32R, kind="ExternalInput")
    out_d = nc.dram_tensor("out_t", [OUT_M, 128, NT], F32, kind="ExternalOutput")

    mult = mybir.AluOpType.mult

    with tile.TileContext(nc) as tc:
        with tc.tile_pool(name="singles", bufs=1) as singles:
            # long-lived tensors
            cos_sb = singles.tile([128, NT], F32)
            nc.sync.dma_start(out=cos_sb, in_=cos_d[:, :])
            sin_sb = singles.tile([128, NT], F32)
            nc.sync.dma_start(out=sin_sb, in_=sin_d[:, :])
            bq_sb = singles.tile([128, QKV_M], F32)
            nc.sync.dma_start(out=bq_sb, in_=bqkv_d[:, :])
            sink_sb = singles.tile([1, 4 * CH], F32)
            nc.sync.dma_start(out=sink_sb, in_=sink_d[:, :])
            id_sb = singles.tile([128, 64], F32)
            nc.sync.dma_start(out=id_sb, in_=id_d[:, :])
            ones_sb = singles.tile([128, 1], F32R)
            nc.sync.dma_start(out=ones_sb, in_=ones_d[:, :])
            eps_sb = singles.tile([1, 1], F32)
            nc.vector.memset(eps_sb, EPS)

            q_sb = singles.tile([128, 8 * NT], F32R)     # 8 head-pair tiles
            k_sb = singles.tile([128, NT], F32R)
            v_sb = singles.tile([128, NT], F32)
            vt_sb = singles.tile([128, 16 * 65], F32R)   # (g,kt) -> [128k, 64d + ones]
            rrms_sb = singles.tile([1, NT], F32)

            for t in range(16):                         # ones column per (g,kt) tile
                nc.sync.dma_start(out=vt_sb[:, t * 65 + 64:t * 65 + 65], in_=ones_d[:, :])

            # ---------------- phase A: rms stats + qkv + rope + v transpose
            with (
                tc.tile_pool(name="xtp", bufs=1) as xtp,
                tc.tile_pool(name="wqp", bufs=2) as wqp,
                tc.tile_pool(name="sqp", bufs=2) as sqp,
                tc.tile_pool(name="ropep", bufs=2) as ropep,
                tc.tile_pool(name="ps_ssq", bufs=2, space="PSUM") as ps_ssq,
                tc.tile_pool(name="ps_mm", bufs=4, space="PSUM") as ps_mm,
                tc.tile_pool(name="ps_vt", bufs=1, space="PSUM") as ps_vt,
            ):
                xt_sb = xtp.tile([128, KT * NT], F32R)
                rrms128 = xtp.tile([128, NT], F32)
                for k in range(KT):
                    nc.sync.dma_start(
                        out=xt_sb[:, k * NT:(k + 1) * NT], in_=xt_d[k, :, :]
                    )

                # sum of squares over hidden via ones-matmul, then 1/sqrt
                for c in range(2):
                    ssq_ps = ps_ssq.tile([1, CH], F32, tag="ssq")
                    for k in range(KT):
                        sq = sqp.tile([128, CH], F32R, tag="sq")
                        nc.scalar.square(
                            out=sq, in_=xt_sb[:, k * NT + c * CH: k * NT + (c + 1) * CH]
                        )
                        nc.tensor.matmul(
                            ssq_ps,
                            ones_sb,
                            sq,
                            start=(k == 0),
                            stop=(k == KT - 1),
                        )
                    rr = rrms_sb[0:1, c * CH:(c + 1) * CH]
                    nc.scalar.activation(
                        out=rr, in_=ssq_ps,
                        func=mybir.ActivationFunctionType.Sqrt,
                        bias=eps_sb[0:1, 0:1], scale=1.0 / HID,
                    )
                    nc.vector.reciprocal(out=rr, in_=rr)
                # materialize the per-token 1/rms broadcast across partitions
                nc.gpsimd.partition_broadcast(rrms128, rrms_sb[0:1, :])

                def qkv_dest(m):
                    if m == 8:
                        return k_sb
                    if m == 9:
                        return v_sb
                    return q_sb[:, m * NT:(m + 1) * NT]

                def rope(dest):
                    # dest: [128, NT] head-pair tile, pairs interleaved
                    for c in range(2):
                        sl = slice(c * CH, (c + 1) * CH)
                        sh = ropep.tile([128, CH], F32, tag="rope")
                        nc.vector.stream_shuffle(out=sh, in_=dest[:, sl].bitcast(F32), mask=PAIR_SWAP)
                        nc.vector.tensor_mul(sh, sh, sin_sb[:, sl])
                        nc.vector.tensor_mul(dest[:, sl], dest[:, sl], cos_sb[:, sl])
                        nc.vector.tensor_add(dest[:, sl], dest[:, sl], sh)

                for m in (8, 9, 0, 1, 2, 3, 4, 5, 6, 7):
                    wq_sb = wqp.tile([128, KT * 128], F32R, tag="wq")
                    nc.sync.dma_start(out=wq_sb, in_=wqkv_d[m, :, :])
                    dest = qkv_dest(m)
                    ps = [ps_mm.tile([128, CH], F32, tag="mm", name=f"mm{m}_{i}") for i in range(2)]
                    for k in range(KT):
                        lhsT = wq_sb[:, k * 128:(k + 1) * 128]
                        for c in range(2):
                            nc.tensor.matmul(
                                ps[c],
                                lhsT,
                                xt_sb[:, k * NT + c * CH: k * NT + (c + 1) * CH],
                                start=(k == 0),
                                stop=(k == KT - 1),
                            )
                    for c in range(2):
                        dsl = dest[:, c * CH:(c + 1) * CH]
                        nc.vector.tensor_tensor(
                            out=dsl, in0=ps[c],
                            in1=rrms128[:, c * CH:(c + 1) * CH],
                            op=mult,
                        )
                        nc.scalar.activation(
                            out=dsl, in_=dsl,
                            func=mybir.ActivationFunctionType.Identity,
                            bias=bq_sb[:, m:m + 1], scale=1.0,
                        )
                    if m == 8:
                        rope(k_sb)
                    elif m == 9:
                        # transpose v into [k, d] tiles (+ ones col kept from memset)
                        for g in range(2):
                            for kt in range(8):
                                pst = ps_vt.tile([128, 64], F32, tag="vt")
                                nc.tensor.matmul(
                                    pst,
                                    v_sb[g * 64:(g + 1) * 64, kt * 128:(kt + 1) * 128],
                                    id_sb[g * 64:(g + 1) * 64, :],
                                    is_transpose=True,
                                    start=True, stop=True,
                                )
                                nc.vector.tensor_copy(
                                    out=vt_sb[:, (g * 8 + kt) * 65:(g * 8 + kt) * 65 + 64],
                                    in_=pst,
                                )
                    else:
                        rope(qkv_dest(m))

            # ---------------- phase B: attention + output projection
            with (
                tc.tile_pool(name="attnp", bufs=1) as attnp,
                tc.tile_pool(name="maskp", bufs=1) as maskp,
                tc.tile_pool(name="wexp", bufs=4) as wexp,
                tc.tile_pool(name="dnp", bufs=4) as dnp,
                tc.tile_pool(name="wop", bufs=2) as wop,
                tc.tile_pool(name="otp", bufs=3) as otp,
                tc.tile_pool(name="ps_att", bufs=4, space="PSUM") as ps_att,
                tc.tile_pool(name="ps_pv", bufs=2, space="PSUM") as ps_pv,
                tc.tile_pool(name="ps_o", bufs=2, space="PSUM") as ps_o,
            ):
                maskd_sb = maskp.tile([128, CH], F32)
                nc.sync.dma_start(out=maskd_sb, in_=maskd_d[:, :])
                maskl_sb = maskp.tile([128, CH], F32)
                nc.sync.dma_start(out=maskl_sb, in_=maskl_d[:, :])
                attn_sb = attnp.tile([128, 8 * NT], F32R)

                q_v = q_sb.rearrange("p (h t) -> p h t", t=NT)
                a_v = attn_sb.rearrange("p (h t) -> p h t", t=NT)

                for g in range(2):
                    prng = slice(g * 64, (g + 1) * 64)
                    for qt in range(8):
                        kts = [qt] if qt == 0 else [qt - 1, qt]
                        for a in range(2):
                            rhs_q = q_v[prng, 4 * a:4 * a + 4, qt * 128:(qt + 1) * 128]
                            ws = []
                            for kt in kts:
                                psl = ps_att.tile([128, CH], F32, tag="l")
                                nc.tensor.matmul(
                                    psl,
                                    k_sb[prng, kt * 128:(kt + 1) * 128],
                                    rhs_q,
                                    start=True, stop=True,
                                )
                                w = wexp.tile([128, CH], F32R, tag="w")
                                nc.vector.tensor_add(
                                    w, psl, maskd_sb if kt == qt else maskl_sb
                                )
                                nc.scalar.activation(
                                    out=w, in_=w, func=mybir.ActivationFunctionType.Exp
                                )
                                ws.append((kt, w))
                            pspv = ps_pv.tile([65, CH], F32, tag="pv")
                            for i, (kt, w) in enumerate(ws):
                                nc.tensor.matmul(
                                    pspv,
                                    vt_sb[:, (g * 8 + kt) * 65:(g * 8 + kt + 1) * 65],
                                    w,
                                    start=(i == 0),
                                    stop=(i == len(ws) - 1),
                                )
                            dn = dnp.tile([1, CH], F32, tag="dn")
                            nc.vector.tensor_add(
                                dn, pspv[64:65, :], sink_sb[0:1, (2 * g + a) * CH:(2 * g + a + 1) * CH]
                            )
                            nc.vector.reciprocal(out=dn, in_=dn)
                            dnb = dnp.tile([64, CH], F32, tag="dnb")
                            nc.gpsimd.partition_broadcast(dnb, dn)
                            nc.vector.tensor_tensor(
                                out=a_v[prng, 4 * a:4 * a + 4, qt * 128:(qt + 1) * 128],
                                in0=pspv[0:64, :],
                                in1=dnb,
                                op=mult,
                            )

                for m in range(OUT_M):
                    wo_sb = wop.tile([128, OUT_K * 128], F32R, tag="wo")
                    nc.sync.dma_start(out=wo_sb, in_=wout_d[m, :, :])
                    ps = [ps_o.tile([128, CH], F32, tag="o", name=f"o{m}_{i}") for i in range(2)]
                    for k in range(OUT_K):
                        lhsT = wo_sb[:, k * 128:(k + 1) * 128]
                        for c in range(2):
                            nc.tensor.matmul(
                                ps[c],
                                lhsT,
                                attn_sb[:, k * NT + c * CH: k * NT + (c + 1) * CH],
                                start=(k == 0),
                                stop=(k == OUT_K - 1),
                            )
                    for c in range(2):
                        ot = otp.tile([128, CH], F32, tag="ot")
                        nc.vector.tensor_copy(out=ot, in_=ps[c])
                        nc.sync.dma_start(
                            out=out_d[m, :, c * CH:(c + 1) * CH], in_=ot
                        )

    nc.compile()
    return nc


# ---------------------------------------------------------------- host prep
def _rope_tables():
    # verbatim fp32 port of the reference YaRN cache
    steps = np.arange(0, 64, 2, dtype=np.float32)
    freq = np.power(np.float32(150000.0), steps / np.float32(64))
    conc = np.float32(0.1) * np.log(np.float32(32.0)) + 1.0
    d_half = np.float32(32.0)
    log_base = np.log(np.float32(150000.0))
    low = d_half * np.log(np.float32(4096) / (np.float32(32.0) * np.float32(2.0 * np.pi))) / log_base
    high = d_half * np.log(np.float32(4096) / (np.float32(1.0) * np.float32(2.0 * np.pi))) / log_base
    ramp = (np.arange(32, dtype=np.float32) - low) / (high - low)
    mask = 1.0 - np.clip(ramp, 0.0, 1.0)
    inv_freq = (1.0 / (np.float32(32.0) * freq)) * (1.0 - mask) + (1.0 / freq) * mask
    pos = np.arange(SEQ, dtype=np.float32)
    freqs = np.einsum("i,j->ij", pos, inv_freq.astype(np.float32))
    cos = (np.cos(freqs) * conc).astype(np.float32)  # (SEQ, 32)
    sin = (np.sin(freqs) * conc).astype(np.float32)
    return cos, sin


_ILV = np.empty(64, np.int64)
_ILV[0::2] = np.arange(32)
_ILV[1::2] = np.arange(32) + 32


def _interleave_rows(w):
    # w: (n_heads*64, ...) -> rope-pair-interleaved rows per 64-row head block
    nh = w.shape[0] // 64
    idx = (np.arange(nh)[:, None] * 64 + _ILV[None, :]).reshape(-1)
    return w[idx]


def prep_inputs(x, norm_w, qkv_w, qkv_b, out_w, sinks):
    x = np.asarray(x, np.float32)
    norm_w = np.asarray(norm_w, np.float32)
    qkv_w = np.asarray(qkv_w, np.float32)
    qkv_b = np.asarray(qkv_b, np.float32)
    out_w = np.asarray(out_w, np.float32)
    sinks = np.asarray(sinks, np.float32)

    cos, sin = _rope_tables()
    cosT, sinT = cos.T, sin.T                      # (32, SEQ)
    cos64 = np.repeat(cosT, 2, axis=0)             # lo/hi both use cos_i
    sin64 = np.repeat(sinT, 2, axis=0).copy()
    sin64[0::2] *= -1.0                            # lo gets -sin
    cos128 = np.ascontiguousarray(np.concatenate([cos64, cos64], axis=0))
    sin128 = np.ascontiguousarray(np.concatenate([sin64, sin64], axis=0))

    i = np.arange(128)[:, None]
    j = np.arange(128)[None, :]
    maskd = np.where(i <= j, 0.0, MASK_NEG).astype(np.float32)
    maskl = np.where(i > j, 0.0, MASK_NEG).astype(np.float32)
    maskd = np.ascontiguousarray(np.tile(maskd, (1, 4)))
    maskl = np.ascontiguousarray(np.tile(maskl, (1, 4)))

    eye = np.eye(64, dtype=np.float32)
    ident64 = np.ascontiguousarray(np.concatenate([eye, eye], axis=0))  # (128, 64)

    w_eff = qkv_w * norm_w[None, :]
    b_eff = qkv_b.copy()
    w_eff[:NH * D] *= 0.125
    b_eff[:NH * D] *= 0.125

    in_maps = []
    for c in range(8):
        b, g2 = divmod(c, 4)
        # Q m-tile m holds heads (16*g2+m) [partitions 0:64] and (16*g2+8+m)
        # [partitions 64:128], rope-pair interleaved within each head.
        qheads = np.empty(16, np.int64)
        qheads[0::2] = 16 * g2 + np.arange(8)        # g=0 heads, even slots
        qheads[1::2] = 16 * g2 + 8 + np.arange(8)    # g=1 heads, odd slots
        qrows = (qheads[:, None] * D + _ILV[None, :]).reshape(-1)
        krows = NH * D + np.arange(2 * g2 * D, 2 * (g2 + 1) * D)
        vrows = (NH + NKV) * D + np.arange(2 * g2 * D, 2 * (g2 + 1) * D)
        krows = krows.reshape(2, 64)[:, _ILV].reshape(-1)
        rowsel = np.concatenate([qrows, krows, vrows])
        Wc = w_eff[rowsel]                          # (1280, 2880)
        bc = b_eff[rowsel]

        WcT = np.zeros((HIDP, 1280), np.float32)
        WcT[:HID] = Wc.T
        wqkv = np.ascontiguousarray(
            WcT.reshape(KT, 128, QKV_M, 128).transpose(2, 1, 0, 3).reshape(QKV_M, 128, KT * 128)
        )
        bqkv = np.ascontiguousarray(bc.reshape(QKV_M, 128).T)

        # attn feature f: tile ft=f//128, partition p=f%128 -> g=p//64, hq=ft
        f = np.arange(1024)
        colsel = (16 * g2 + 8 * ((f % 128) // 64) + f // 128) * D + (f % 64)
        WoT = np.zeros((1024, HIDP), np.float32)
        WoT[:, :HID] = out_w[:, colsel].T
        wout = np.ascontiguousarray(
            WoT.reshape(OUT_K, 128, OUT_M, 128).transpose(2, 1, 0, 3).reshape(OUT_M, 128, OUT_K * 128)
        )

        xp = np.zeros((HIDP, NT), np.float32)
        xp[:HID] = x[b].T
        xt = np.ascontiguousarray(xp.reshape(KT, 128, NT))

        sinkrow = np.empty((1, 4 * CH), np.float32)
        for g in range(2):
            for a in range(2):
                hl = 8 * g + 4 * a + np.arange(4)        # local heads per quad
                se = np.exp(sinks[16 * g2 + hl].astype(np.float32))
                sinkrow[0, (2 * g + a) * CH:(2 * g + a + 1) * CH] = np.repeat(se, 128)

        in_maps.append({
            "xt": xt, "wqkv": wqkv, "bqkv": bqkv, "wout": wout,
            "cos128": cos128, "sin128": sin128,
            "maskd": maskd, "maskl": maskl,
            "sinkrow": sinkrow, "ident64": ident64,
            "ones128": np.ones((128, 1), np.float32),
        })
    return in_maps


def unshard(results, x, out_b):
    x = np.asarray(x, np.float32)
    out_b = np.asarray(out_b, np.float32)
    y = np.empty((B, SEQ, HID), np.float32)
    for b in range(B):
        acc = np.zeros((HIDP, NT), np.float64)
        for g2 in range(4):
            acc += results[4 * b + g2]["out_t"].reshape(HIDP, NT)
        y[b] = x[b] + acc[:HID].T.astype(np.float32) + out_b[None, :]
    return y


_NC_CACHE = []


def kernel(x, norm_w, qkv_w, qkv_b, out_w, out_b, sinks):
    in_maps = prep_inputs(x, norm_w, qkv_w, qkv_b, out_w, sinks)
    if not _NC_CACHE:
        _NC_CACHE.append(build_nc())
    nc = _NC_CACHE[0]
    res = run_bass_kernel_spmd(nc, in_maps, core_ids=list(range(8)))
    return unshard(res.results, x, out_b)
